# revision 20
# baseline (speedup 1.0000x reference)
"""Trainium2 Bass kernel for nn_FCN_81621558493619.

Computation: 3-layer MLP (mat-vec, 8192->8192->8192->16394) + box-filter +
linear interpolation + Fermi-window trapezoid integrals.

Strategy (8 NeuronCores, SPMD + collectives):
  - Tensor-parallel over output rows of W1/W2/W3 (1024/1024/2176 rows per
    core, W3 zero-padded 16394->17408). Weights bf16, host-permuted so each
    weight DMA is fully contiguous (8KB/partition for W1/W2). Mat-vec on the
    TensorEngine: x chunk [128,1] stationary, weight tiles [128,N<=512]
    moving, 64 k-chunks accumulate in PSUM; bias folded via a final K=1
    matmul against ones. Weight-streaming DMAs own the sync (SP) HWDGE
    queue; DMAs that wait on collectives/compute go on the scalar queue.
  - Interpolation exploits the +-j symmetry of the quadrature (s_j =
    -s_{100-j}, Fermi window even): for j>=50 the positions pos = 8192 +
    x*s_j and their mirrors 16384-pos interpolate from table entries k and
    8191-k with swapped fractions. A packed bf16 table T4[k] = (S10[8192+k],
    S10[8193+k], S10[8191-k], S10[8192-k]) lets ONE ap_gather index serve
    both j and 100-j, halving the dominant gpsimd gather cost (~27ns/idx).
    The mirrored table segments are built with two one-hot permutation
    matmuls on the PE (partition shift/reverse) plus DVE negative-stride
    copies (column reverse). Blend coefficients A0..A3 are precomputed on
    the DVE under the MLP; the tail is a 4-term blend and log2-fold
    reductions for I1/I2. A dummy gather after AllGather 1 absorbs the
    gpsimd ucode-library reload off the critical path.
"""
import numpy as np
import ml_dtypes

import concourse.bacc as bacc
import concourse.mybir as mybir
from concourse import tile
from concourse.ap import AP as _AP
from concourse.bass_utils import run_bass_kernel_spmd

F32 = mybir.dt.float32
BF16 = mybir.dt.bfloat16
I16 = mybir.dt.int16
I32 = mybir.dt.int32

SIZE = 8192
NCORE = 8
SH1 = SIZE // NCORE            # 1024 rows of W1/W2 per core
NROW3 = 2 * SIZE + 10          # 16394
NROW3P = 17408                 # padded to 8*2176
SH3 = NROW3P // NCORE          # 2176
KC = 64                        # k-chunks of 128 along the contraction dim
KB = 4                         # k-chunks per weight DMA
NG = KC // KB                  # 16 weight-DMA groups per layer
NJ2 = 51                       # j = 50..100 (mirror covers 0..49)
NU = 128                       # samples per 16-partition group
NQ = NJ2 * NU                  # 6528 gather indices per core
PJ = [(0, 13), (13, 26), (26, 39), (39, 51)]   # prep jj-chunks


def build_nc():
    nc = bacc.Bacc("TRN2", target_bir_lowering=False, debug=False,
                   num_devices=NCORE)

    # ---- per-core external inputs ----
    w1 = nc.dram_tensor("w1", [NG, 128, KB, SH1], BF16, kind="ExternalInput")
    w2 = nc.dram_tensor("w2", [NG, 128, KB, SH1], BF16, kind="ExternalInput")
    w3 = nc.dram_tensor("w3", [NG, 128, KB, SH3], BF16, kind="ExternalInput")
    b1d = nc.dram_tensor("b1d", [SH1], BF16, kind="ExternalInput")
    b2d = nc.dram_tensor("b2d", [SH1], BF16, kind="ExternalInput")
    b3d = nc.dram_tensor("b3d", [SH3], BF16, kind="ExternalInput")
    xsd = nc.dram_tensor("xsd", [128, KC], BF16, kind="ExternalInput")
    xu = nc.dram_tensor("xu", [128, NU], F32, kind="ExternalInput")
    xg = nc.dram_tensor("xg", [128, 8], F32, kind="ExternalInput")
    sjb = nc.dram_tensor("sjb", [128, NJ2], F32, kind="ExternalInput")
    abF = nc.dram_tensor("abF", [128, NJ2], F32, kind="ExternalInput")
    abM = nc.dram_tensor("abM", [128, NJ2], F32, kind="ExternalInput")
    tnb = nc.dram_tensor("tnb", [128, NJ2], BF16, kind="ExternalInput")
    # one-hot permutations: permd[k,m] = (k==m+64), permr[k,m] = (k==63-m)
    permd = nc.dram_tensor("permd", [128, 128], BF16, kind="ExternalInput")
    permr = nc.dram_tensor("permr", [128, 128], BF16, kind="ExternalInput")
    out = nc.dram_tensor("out", [2048], F32, kind="ExternalOutput")

    RG = [list(range(NCORE))]

    with tile.TileContext(nc) as tc:
        with tc.tile_pool(name="dram", bufs=1, space="DRAM") as dpool, \
             tc.tile_pool(name="small", bufs=1) as sp:
            # persistent small tiles
            xut = sp.tile([128, NU], F32)
            xgt = sp.tile([128, 8], F32)
            sjt = sp.tile([128, NJ2], F32)
            abFt = sp.tile([128, NJ2], F32)
            abMt = sp.tile([128, NJ2], F32)
            tnt = sp.tile([128, NJ2], BF16)
            pdt = sp.tile([128, 128], BF16)
            prt = sp.tile([128, 128], BF16)
            idxt = sp.tile([128, NJ2 * 8], I16)
            A0 = sp.tile([128, NQ], BF16)   # a_j*(1-frac)
            A1 = sp.tile([128, NQ], BF16)   # a_j*frac
            A2 = sp.tile([128, NQ], BF16)   # a_mir*frac
            A3 = sp.tile([128, NQ], BF16)   # a_mir*(1-frac)
            i1acc = sp.tile([128, NU], F32)
            i2acc = sp.tile([128, NU], F32)
            ones1 = sp.tile([1, 1], BF16)
            dtab = sp.tile([128, 64], BF16)
            didx = sp.tile([128, 4], I16)
            dout = sp.tile([128, 64], BF16)

            # DRAM bounce buffers
            cc_in1 = dpool.tile([SH1], BF16, name="cc_in1")
            cc_out1 = dpool.tile([SIZE], BF16, name="cc_out1")
            cc_in2 = dpool.tile([SH1], BF16, name="cc_in2")
            cc_out2 = dpool.tile([SIZE], BF16, name="cc_out2")
            q_in = dpool.tile([SH3], BF16, name="q_in")
            q_full = dpool.tile([NROW3P], BF16, name="q_full")
            t4d = dpool.tile([4 * SIZE], BF16, name="t4d")  # 8192 x d=4

            # ---------------- index / coefficient prep (DVE, under MLP) ---
            def do_prep(pp):
                for j0, j1 in PJ:
                    jc = j1 - j0
                    wq = jc * NU
                    sx = pp.tile([128, 13 * NU], F32, tag="sx",
                                 name="sx")[:, 0:wq]
                    pm = pp.tile([128, 13 * NU], F32, tag="pm",
                                 name="pm")[:, 0:wq]
                    i0i = pp.tile([128, 13 * NU], I32, tag="i0i",
                                  name="i0i")[:, 0:wq]
                    i0f = pp.tile([128, 13 * NU], F32, tag="i0f",
                                  name="i0f")[:, 0:wq]
                    frc = pp.tile([128, 13 * NU], BF16, tag="frc",
                                  name="frc")[:, 0:wq]
                    # sx[p, (jj,u)] = s_j * x_u  (k = floor(sx))
                    nc.vector.tensor_tensor(
                        out=sx[:],
                        in0=xut[:].unsqueeze(1).to_broadcast([128, jc, NU]),
                        in1=sjt[:, j0:j1].unsqueeze(2).to_broadcast(
                            [128, jc, NU]),
                        op=mybir.AluOpType.mult)
                    nc.vector.tensor_scalar_add(pm[:], sx[:], -0.5)
                    nc.vector.tensor_copy(i0i[:], pm[:])  # cast rounds->floor
                    # i0f = min(k, 8191)  (f32)
                    nc.vector.tensor_scalar(
                        out=i0f[:], in0=i0i[:], scalar1=SIZE - 1, scalar2=0,
                        op0=mybir.AluOpType.min, op1=mybir.AluOpType.max)
                    # frac = sx - k
                    nc.vector.tensor_tensor(
                        out=frc[:], in0=sx[:], in1=i0f[:],
                        op=mybir.AluOpType.subtract)
                    abFv = abFt[:, j0:j1].unsqueeze(2).to_broadcast(
                        [128, jc, NU])
                    abMv = abMt[:, j0:j1].unsqueeze(2).to_broadcast(
                        [128, jc, NU])
                    sl = slice(j0 * NU, j1 * NU)
                    nc.vector.tensor_tensor(
                        out=A1[:, sl], in0=frc[:], in1=abFv,
                        op=mybir.AluOpType.mult)
                    nc.vector.tensor_tensor(
                        out=A0[:, sl], in0=abFv, in1=A1[:, sl],
                        op=mybir.AluOpType.subtract)
                    nc.vector.tensor_tensor(
                        out=A2[:, sl], in0=frc[:], in1=abMv,
                        op=mybir.AluOpType.mult)
                    nc.vector.tensor_tensor(
                        out=A3[:, sl], in0=abMv, in1=A2[:, sl],
                        op=mybir.AluOpType.subtract)

                    # compact index path for the gather (8 idx per j)
                    wg = jc * 8
                    sxg = pp.tile([128, 13 * 8], F32, tag="sxg",
                                  name="sxg")[:, 0:wg]
                    pmg = pp.tile([128, 13 * 8], F32, tag="pmg",
                                  name="pmg")[:, 0:wg]
                    i0g = pp.tile([128, 13 * 8], I32, tag="i0g",
                                  name="i0g")[:, 0:wg]
                    nc.vector.tensor_tensor(
                        out=sxg[:],
                        in0=xgt[:].unsqueeze(1).to_broadcast([128, jc, 8]),
                        in1=sjt[:, j0:j1].unsqueeze(2).to_broadcast(
                            [128, jc, 8]),
                        op=mybir.AluOpType.mult)
                    nc.vector.tensor_scalar_add(pmg[:], sxg[:], -0.5)
                    nc.vector.tensor_copy(i0g[:], pmg[:])
                    nc.vector.tensor_scalar(
                        out=idxt[:, j0 * 8:j1 * 8], in0=i0g[:],
                        scalar1=SIZE - 1, scalar2=0,
                        op0=mybir.AluOpType.min, op1=mybir.AluOpType.max)

            # ---------------- MLP on the TensorEngine ----------------
            def matvec_layer(wdram, wpool, sh, xs_t, bv_t, psums, wtag):
                for g in range(NG):
                    wt = wpool.tile([128, KB, sh], BF16, tag="w",
                                    name=f"{wtag}_{g}")
                    nc.sync.dma_start(wt[:], wdram[g])
                    for ki in range(KB):
                        kc = g * KB + ki
                        for ps, n0, n1 in psums:
                            nc.tensor.matmul(
                                ps[:], xs_t[:, kc:kc + 1], wt[:, ki, n0:n1],
                                start=(kc == 0), stop=False)
                for ps, n0, n1 in psums:
                    nc.tensor.matmul(
                        ps[:], ones1[:], bv_t[:, n0:n1],
                        start=False, stop=True)

            with tc.tile_pool(name="w12", bufs=5) as wp12, \
                 tc.tile_pool(name="w3p", bufs=3) as wp3, \
                 tc.tile_pool(name="mlp_misc", bufs=1) as mp, \
                 tc.tile_pool(name="prep", bufs=1) as pp, \
                 tc.psum_pool(name="mlp_ps", bufs=1) as pspool:
                xs1 = mp.tile([128, KC], BF16)
                b1v = mp.tile([1, SH1], BF16)
                h1v = mp.tile([1, SH1], BF16)
                nc.sync.dma_start(xs1[:], xsd[:])
                nc.scalar.dma_start(b1v[:], b1d.ap()[None, :])
                nc.scalar.dma_start(xut[:], xu[:])
                nc.scalar.dma_start(xgt[:], xg[:])
                nc.scalar.dma_start(sjt[:], sjb[:])
                nc.scalar.dma_start(abFt[:], abF[:])
                nc.scalar.dma_start(abMt[:], abM[:])
                nc.scalar.dma_start(tnt[:], tnb[:])
                nc.scalar.dma_start(pdt[:], permd[:])
                nc.scalar.dma_start(prt[:], permr[:])
                nc.vector.memset(ones1[:], 1.0)
                nc.vector.memset(dtab[:], 0.0)
                nc.vector.memset(didx[:], 0)

                ps1 = [pspool.tile([1, 512], F32, tag="ps", bufs=5,
                                   name=f"ps1_{i}")
                       for i in range(2)]
                blocks1 = [(ps1[0], 0, 512), (ps1[1], 512, 1024)]
                matvec_layer(w1, wp12, SH1, xs1, b1v, blocks1, "w1")
                do_prep(pp)
                for ps, n0, n1 in blocks1:
                    nc.scalar.activation(
                        out=h1v[:, n0:n1], in_=ps[:],
                        func=mybir.ActivationFunctionType.Relu)
                nc.scalar.dma_start(cc_in1[:].rearrange("(o n) -> o n", o=1),
                                    h1v[:])
                nc.gpsimd.collective_compute(
                    "AllGather", mybir.AluOpType.bypass, replica_groups=RG,
                    ins=[cc_in1.opt()], outs=[cc_out1.opt()])
                # dummy gather: loads the ap_gather ucode library off the
                # critical path (hidden under layer 2)
                nc.gpsimd.ap_gather(dout[:, 0:32], dtab[:, 0:32],
                                    didx[:], channels=128, num_elems=8,
                                    d=4, num_idxs=8)

                xs2 = mp.tile([128, KC], BF16)
                b2v = mp.tile([1, SH1], BF16)
                h2v = mp.tile([1, SH1], BF16)
                nc.scalar.dma_start(
                    xs2[:], cc_out1[:].rearrange("(p c) -> p c", p=128))
                nc.scalar.dma_start(b2v[:], b2d.ap()[None, :])
                ps2 = [pspool.tile([1, 512], F32, tag="ps", bufs=5,
                                   name=f"ps2_{i}")
                       for i in range(2)]
                blocks2 = [(ps2[0], 0, 512), (ps2[1], 512, 1024)]
                matvec_layer(w2, wp12, SH1, xs2, b2v, blocks2, "w2")
                for ps, n0, n1 in blocks2:
                    nc.scalar.activation(
                        out=h2v[:, n0:n1], in_=ps[:],
                        func=mybir.ActivationFunctionType.Relu)
                nc.scalar.dma_start(cc_in2[:].rearrange("(o n) -> o n", o=1),
                                    h2v[:])
                nc.gpsimd.collective_compute(
                    "AllGather", mybir.AluOpType.bypass, replica_groups=RG,
                    ins=[cc_in2.opt()], outs=[cc_out2.opt()])

                xs3 = mp.tile([128, KC], BF16)
                b3v = mp.tile([1, SH3], BF16)
                q3v = mp.tile([1, SH3], BF16)
                nc.scalar.dma_start(
                    xs3[:], cc_out2[:].rearrange("(p c) -> p c", p=128))
                nc.scalar.dma_start(b3v[:], b3d.ap()[None, :])
                ps3 = [pspool.tile([1, 512], F32, tag="ps", bufs=5,
                                   name=f"ps3_{i}")
                       for i in range(4)]
                ps3e = pspool.tile([1, 128], F32, tag="pse", bufs=1,
                                   name="ps3_4")
                blocks3 = [(ps3[i], 512 * i, 512 * (i + 1)) for i in range(4)]
                blocks3.append((ps3e, 2048, 2176))
                matvec_layer(w3, wp3, SH3, xs3, b3v, blocks3, "w3")
                for ps, n0, n1 in blocks3:
                    nc.scalar.activation(
                        out=q3v[:, n0:n1], in_=ps[:],
                        func=mybir.ActivationFunctionType.Copy)
                nc.scalar.dma_start(q_in[:].rearrange("(o n) -> o n", o=1),
                                    q3v[:])
                nc.gpsimd.collective_compute(
                    "AllGather", mybir.AluOpType.bypass, replica_groups=RG,
                    ins=[q_in.opt()], outs=[q_full.opt()])

            # ---------------- box sum -> packed symmetric table -----------
            with tc.tile_pool(name="ph2", bufs=1) as ph2:
                gab = ph2.tile([128, 4 * NQ], BF16)
                with tc.tile_pool(name="sig", bufs=1) as gp, \
                     tc.psum_pool(name="ph2_ps", bufs=1) as pq:
                    qov = gp.tile([128, 144], BF16)
                    sig = gp.tile([128, 129], F32)
                    qf_ap = q_full[:]
                    nc.scalar.dma_start(
                        qov[:], _AP(qf_ap.tensor, 0, [[128, 128], [1, 144]]))
                    u1 = gp.tile([128, 139], F32)
                    u2 = gp.tile([128, 133], F32)
                    nc.vector.tensor_tensor(out=u1[:], in0=qov[:, 0:139],
                                            in1=qov[:, 1:140],
                                            op=mybir.AluOpType.add)
                    nc.vector.tensor_tensor(out=u2[:], in0=u1[:, 0:133],
                                            in1=u1[:, 2:135],
                                            op=mybir.AluOpType.add)
                    nc.vector.tensor_tensor(out=sig[:], in0=u2[:, 0:129],
                                            in1=u2[:, 4:133],
                                            op=mybir.AluOpType.add)
                    nc.vector.tensor_tensor(out=sig[:], in0=sig[:],
                                            in1=u1[:, 8:137],
                                            op=mybir.AluOpType.add)
                    # bf16 copy, then permutation matmuls:
                    # csb[p,c] = sigb[p+64,c]; dsb[p,c] = sigb[63-p,c]
                    sigb = gp.tile([128, 129], BF16)
                    nc.vector.tensor_copy(sigb[:], sig[:])
                    cs_ps = pq.tile([128, 129], F32, tag="cs", name="cs_ps")
                    ds_ps = pq.tile([128, 129], F32, tag="ds", name="ds_ps")
                    nc.tensor.matmul(cs_ps[:], pdt[:], sigb[:],
                                     start=True, stop=True)
                    nc.tensor.matmul(ds_ps[:], prt[:], sigb[:],
                                     start=True, stop=True)
                    csb = gp.tile([128, 129], BF16)
                    dsb = gp.tile([128, 129], BF16)
                    nc.vector.tensor_copy(csb[:], cs_ps[:])
                    nc.vector.tensor_copy(dsb[:], ds_ps[:])
                    # pack T4 on 64 partitions: t4_sb[p, 4c+s]
                    t4_sb = gp.tile([64, 512], BF16)
                    t4v = t4_sb[:].rearrange("p (c s) -> p c s", s=4)
                    nc.vector.tensor_copy(t4v[:, :, 0], csb[0:64, 0:128])
                    nc.vector.tensor_copy(t4v[:, :, 1], csb[0:64, 1:129])
                    pstride = dsb[:].ap[0][0]
                    dr0 = _AP(dsb[:].tensor, dsb[:].offset + 127,
                              [[pstride, 64], [-1, 128]])
                    dr1 = _AP(dsb[:].tensor, dsb[:].offset + 128,
                              [[pstride, 64], [-1, 128]])
                    nc.vector.tensor_copy(t4v[:, :, 2], dr0)
                    nc.vector.tensor_copy(t4v[:, :, 3], dr1)
                    nc.scalar.dma_start(
                        t4d[:].rearrange("(p f) -> p f", p=64), t4_sb[:])

                    # broadcast to all partitions, two halves on two queues
                    tab = gp.tile([128, 4 * SIZE], BF16)
                    nc.scalar.dma_start(
                        tab[:, 0:2 * SIZE],
                        t4d[0:2 * SIZE][None, :].to_broadcast(
                            [128, 2 * SIZE]))
                    nc.sync.dma_start(
                        tab[:, 2 * SIZE:4 * SIZE],
                        t4d[2 * SIZE:4 * SIZE][None, :].to_broadcast(
                            [128, 2 * SIZE]))

                    # ---- gather in two j-halves (blend overlaps half 2) --
                    for jj0, jj1 in PJ:
                        nc.gpsimd.ap_gather(
                            gab[:, 4 * jj0 * NU:4 * jj1 * NU], tab[:],
                            idxt[:, jj0 * 8:jj1 * 8],
                            channels=128, num_elems=SIZE, d=4,
                            num_idxs=(jj1 - jj0) * NU)

                # ---------------- blend + fold reductions -----------------
                with tc.tile_pool(name="blend", bufs=1) as bp:
                    gq = gab[:].rearrange("p (q s) -> p q s", s=4)
                    paF = bp.tile([128, NQ], BF16)
                    paR = bp.tile([128, NQ], BF16)
                    tmp = bp.tile([128, NQ], BF16)
                    fs = bp.tile([128, 32 * NU], F32)
                    for jj0, jj1 in PJ:
                        sl = slice(jj0 * NU, jj1 * NU)
                        gh = gq[:, jj0 * NU:jj1 * NU, :]
                        nc.vector.tensor_tensor(out=paF[:, sl],
                                                in0=gh[:, :, 0],
                                                in1=A0[:, sl],
                                                op=mybir.AluOpType.mult)
                        nc.vector.tensor_tensor(out=tmp[:, sl],
                                                in0=gh[:, :, 1],
                                                in1=A1[:, sl],
                                                op=mybir.AluOpType.mult)
                        nc.vector.tensor_tensor(out=paF[:, sl],
                                                in0=paF[:, sl],
                                                in1=tmp[:, sl],
                                                op=mybir.AluOpType.add)
                        nc.vector.tensor_tensor(out=paR[:, sl],
                                                in0=gh[:, :, 2],
                                                in1=A2[:, sl],
                                                op=mybir.AluOpType.mult)
                        nc.vector.tensor_tensor(out=tmp[:, sl],
                                                in0=gh[:, :, 3],
                                                in1=A3[:, sl],
                                                op=mybir.AluOpType.mult)
                        nc.vector.tensor_tensor(out=paR[:, sl],
                                                in0=paR[:, sl],
                                                in1=tmp[:, sl],
                                                op=mybir.AluOpType.add)

                    B = NU

                    def fold51(src, dst_acc):
                        # 51 = 32 + 19
                        nc.vector.tensor_tensor(
                            out=fs[:, 0:19 * B], in0=src[:, 0:19 * B],
                            in1=src[:, 32 * B:51 * B],
                            op=mybir.AluOpType.add)
                        nc.vector.tensor_copy(fs[:, 19 * B:32 * B],
                                              src[:, 19 * B:32 * B])
                        for w in (16, 8, 4, 2, 1):
                            nc.vector.tensor_tensor(
                                out=fs[:, 0:w * B], in0=fs[:, 0:w * B],
                                in1=fs[:, w * B:2 * w * B],
                                op=mybir.AluOpType.add)
                        nc.vector.tensor_copy(dst_acc[:], fs[:, 0:B])

                    # I1 = fold(paF + paR)
                    nc.vector.tensor_tensor(out=tmp[:], in0=paF[:],
                                            in1=paR[:],
                                            op=mybir.AluOpType.add)
                    fold51(tmp, i1acc)
                    # I2 = x * fold(tn_j * (paF - paR))
                    nc.vector.tensor_tensor(out=tmp[:], in0=paF[:],
                                            in1=paR[:],
                                            op=mybir.AluOpType.subtract)
                    t3 = tmp[:].rearrange("p (j u) -> p j u", j=NJ2)
                    nc.vector.tensor_tensor(
                        out=t3, in0=t3,
                        in1=tnt[:].unsqueeze(2).to_broadcast([128, NJ2, NU]),
                        op=mybir.AluOpType.mult)
                    fold51(tmp, i2acc)
                    nc.vector.tensor_tensor(out=i2acc[:], in0=i2acc[:],
                                            in1=xut[:],
                                            op=mybir.AluOpType.mult)
                    nc.scalar.dma_start(
                        out[0:1024].rearrange("(g u) -> g u", g=8, u=NU),
                        i1acc[0:128:16, :])
                    nc.scalar.dma_start(
                        out[1024:2048].rearrange("(g u) -> g u", g=8, u=NU),
                        i2acc[0:128:16, :])

    nc.compile()
    return nc


_NC_CACHE = {}
_LAST_RES = None


def _get_nc():
    if "nc" not in _NC_CACHE:
        _NC_CACHE["nc"] = build_nc()
    return _NC_CACHE["nc"]


def _perm_w(Wshard):
    """[rows, 8192] -> [16 g, 128 p, 4 ki, rows] with k = p*64 + g*4 + ki."""
    r = Wshard.shape[0]
    return np.ascontiguousarray(
        Wshard.T.reshape(128, NG, KB, r).transpose(1, 0, 2, 3))


def _host_prep(x, Wc, W1, b1, W2, b2, W3, b3):
    bf = ml_dtypes.bfloat16
    x = np.asarray(x, np.float32)
    Wcf = np.float64(np.asarray(Wc).item())
    NJ = 101
    t = (np.linspace(-1.0, 1.0, NJ, dtype=np.float32)
         * np.float32(Wcf)).astype(np.float32)
    step = np.float32(Wcf) / np.float32(SIZE)
    s = (t / step).astype(np.float32)           # pos = x*s + SIZE
    eu = np.exp(t.astype(np.float64))
    g = eu / (eu + 1.0) ** 2                     # fermi window * x (x cancels)
    d = np.diff(t.astype(np.float64))            # actual fp32 grid deltas
    wtrap = np.zeros(NJ)
    wtrap[:-1] += 0.5 * d
    wtrap[1:] += 0.5 * d
    a = (0.1 * g * wtrap).astype(np.float32)     # 0.1 = box-filter fold
    # symmetric halves: j = 50..100 forward, mirror weight a_{100-j}
    s2 = s[50:101].copy()
    aF = a[50:101].copy()
    aM = a[50::-1].copy()
    aM[0] = 0.0                                  # j=50 mirror would double-count
    tn2 = (-t[50:101]).astype(np.float32)

    pd_ = np.zeros((128, 128), np.float32)
    pr_ = np.zeros((128, 128), np.float32)
    for m in range(64):
        pd_[m + 64, m] = 1.0                     # csb[m,c] = sigb[m+64,c]
        pr_[63 - m, m] = 1.0                     # dsb[m,c] = sigb[63-m,c]

    W3p = np.zeros((NROW3P, SIZE), dtype=np.float32)
    W3p[:NROW3] = W3
    b3p = np.zeros(NROW3P, dtype=np.float32)
    b3p[:NROW3] = b3

    xbf = x.astype(bf)
    in_maps = []
    for c in range(NCORE):
        xc = x[c * SH1:(c + 1) * SH1]
        xgv = np.zeros((128, 8), np.float32)
        xuv = np.zeros((128, NU), np.float32)
        for gidx in range(8):
            seg = xc[gidx * 128:(gidx + 1) * 128]
            xuv[16 * gidx:16 * (gidx + 1), :] = seg[None, :]
            xgv[16 * gidx:16 * gidx + 16, :] = seg.reshape(8, 16).T
        in_maps.append({
            "w1": _perm_w(W1[c * SH1:(c + 1) * SH1]).astype(bf),
            "w2": _perm_w(W2[c * SH1:(c + 1) * SH1]).astype(bf),
            "w3": _perm_w(W3p[c * SH3:(c + 1) * SH3]).astype(bf),
            "b1d": b1[c * SH1:(c + 1) * SH1].astype(bf),
            "b2d": b2[c * SH1:(c + 1) * SH1].astype(bf),
            "b3d": b3p[c * SH3:(c + 1) * SH3].astype(bf),
            "xsd": xbf.reshape(128, KC),
            "xu": xuv,
            "xg": xgv,
            "sjb": np.broadcast_to(s2, (128, NJ2)).copy(),
            "abF": np.broadcast_to(aF, (128, NJ2)).copy(),
            "abM": np.broadcast_to(aM, (128, NJ2)).copy(),
            "tnb": np.broadcast_to(tn2.astype(bf), (128, NJ2)).copy(),
            "permd": pd_.astype(bf),
            "permr": pr_.astype(bf),
        })
    return in_maps


def kernel(x, Wc, W1, b1, W2, b2, W3, b3, _trace=False, _res_box=None):
    nc = _get_nc()
    in_maps = _host_prep(np.asarray(x), Wc, np.asarray(W1), np.asarray(b1),
                         np.asarray(W2), np.asarray(b2), np.asarray(W3),
                         np.asarray(b3))
    res = run_bass_kernel_spmd(nc, in_maps, core_ids=list(range(NCORE)),
                               trace=_trace)
    global _LAST_RES
    _LAST_RES = res
    if _res_box is not None:
        _res_box.append(res)
    i1 = np.concatenate([r["out"][0:1024] for r in res.results])
    i2 = np.concatenate([r["out"][1024:2048] for r in res.results])
    return np.concatenate([i1, i2]).astype(np.float32)
